# revision 18
# baseline (speedup 1.0000x reference)
"""Trainium2 Bass kernel for nn_CAKernel_47459388621075.

10 steps of x = clip(x + 0.1*relu(conv5x5_circular(x, W)), 0, 1) on
x:(16,3,1024,1024) f32, W:(3,3,5,5) f32.

Sharding: batch-parallel over 8 NeuronCores (2 images/core); the circular
conv is per-image, so no cross-core communication.

Per-core: the whole state lives in SBUF as fp16 for all 10 steps — zero HBM
traffic between steps. Each image is one SBUF tile WIN = [120, 64*514]: 64
row-blocks of R=16 rows, block b occupying free columns [b*514, (b+1)*514)
= column-pair index t in [-1, 513) (1-pair circular halo each side).
Partitions pack (row, channel, column-parity):
  rows v in [0,16):  p = 6v + 2ci + s            (main, [0:96))
  rows -2,-1:        p = 96 + 6(v+2) + 2ci + s   (top halo, [96:108))
  rows 16,17:        p = 108 + 6(v-16) + 2ci + s (bottom halo, [108:120))
Main-first ordering keeps every compute op based at partition 0 (engines
require quadrant-aligned partition bases); halo partitions are written by
DMA only (partition moves need the DMA crossbar; compute engines are
lane-aligned; strided-element DMAs blow the descriptor budget, which is
why the hosts packs/unpacks the pair layout and device I/O is contiguous).

Packing column pairs into partitions doubles the useful work per streamed
PE column: the 5x5x3x3 conv becomes 3 PSUM-accumulated fp16 matmuls per
block (pair-shift delta in {-1,0,+1}) with stationaries S_d[120,96] that
encode (ci,dy,dx,parity) -> (co,r,parity').

Post-conv per block: ACT computes u = relu(psum) (0.1 folded into the
stationaries), DVE adds it into the window in place, Pool clips with
min(.,1) and refreshes the two circular column-pair halo columns. The
2-row halos ride to the neighbour blocks in batched 12-partition DMAs (one
UP + one DOWN DMA per HG consecutive blocks — all windows of an image are
one tile, so the per-block copies merge into a single affine transfer).
Halo DMAs into a window are emitted after that window's matmuls of the
current step, so the step still reads the old halo (Tile's WAR tracking
orders the write after those reads).
"""
import sys

sys.path.insert(0, "/opt/trn_rl_repo")

import numpy as np

N_CORES = 8
R = 16            # output rows per block
KP = 120          # window partitions
MP = 96           # psum partitions
T_OF = 1          # free offset of pair 0 inside a block's window
HG = 4            # halo-DMA batch (blocks per UP/DOWN transfer)
SG = 4            # store batch
LG = 8            # load batch


def _p_of(ci, v, s):
    if 0 <= v < R:
        return 6 * v + 2 * ci + s
    if v < 0:
        return 96 + 6 * (v + 2) + 2 * ci + s
    return 108 + 6 * (v - R) + 2 * ci + s


def make_stationaries(W: np.ndarray) -> np.ndarray:
    """S[d, p, m] for pair-shift d-1 in {-1,0,+1}; 0.1 scale folded in."""
    assert W.shape == (3, 3, 5, 5)
    S = np.zeros((3, KP, MP), dtype=np.float32)
    for co in range(3):
        for ci in range(3):
            for r in range(R):
                for dy in range(5):
                    v = r + dy - 2
                    for dx in range(5):
                        for sp in range(2):
                            tot = sp + dx - 2
                            d = tot >> 1
                            s = tot & 1
                            p = _p_of(ci, v, s)
                            m = 6 * r + 2 * co + sp
                            S[d + 1, p, m] += 0.1 * W[co, ci, dy, dx]
    return S.astype(np.float16)


def pack_x(x: np.ndarray) -> np.ndarray:
    """(n,3,H,W) f32 -> (n, KP, NB, TF) f16 in window layout (p-major)."""
    n, c, H, Wc = x.shape
    NB, T = H // R, Wc // 2
    TF = T + 2
    x16 = x.astype(np.float16)
    xw = np.empty((n, KP, NB, TF), np.float16)
    boff = R * np.arange(NB)
    for w in range(20):
        v = w if w < 16 else (w - 18 if w < 18 else w - 2)  # w16,17 -> -2,-1; w18,19 -> 16,17
        rows = (boff + v) % H
        for ci in range(3):
            for s in range(2):
                p = 6 * w + 2 * ci + s if w < 16 else (
                    96 + 6 * (w - 16) + 2 * ci + s if w < 18 else 108 + 6 * (w - 18) + 2 * ci + s
                )
                xw[:, p, :, T_OF : T_OF + T] = x16[:, ci, rows, s::2]
    xw[:, :, :, 0] = xw[:, :, :, T]
    xw[:, :, :, T + 1] = xw[:, :, :, T_OF]
    return xw


def unpack_y(yw: np.ndarray, H: int, Wc: int) -> np.ndarray:
    """(n, MP, NB, T) f16 window layout -> (n,3,H,W) f32."""
    n = yw.shape[0]
    NB, T = H // R, Wc // 2
    y = np.empty((n, 3, H, Wc), np.float32)
    rows = (R * np.arange(NB)[:, None] + np.arange(R)[None, :])  # (NB, R)
    for v in range(R):
        for ci in range(3):
            for s in range(2):
                p = 6 * v + 2 * ci + s
                y[:, ci, rows[:, v], s::2] = yw[:, p].astype(np.float32)
    return y


def build_body(tc, xw_ap, s_ap, yw_ap, n_img, H, Wc, steps):
    from contextlib import ExitStack

    from concourse import mybir

    nc = tc.nc
    f32 = mybir.dt.float32
    f16 = mybir.dt.float16
    Relu = mybir.ActivationFunctionType.Relu

    NB = H // R
    T = Wc // 2
    TF = T + 2
    assert H % R == 0 and Wc % 2 == 0 and NB >= 3 and NB % HG == 0

    ctx = ExitStack()
    const_pool = ctx.enter_context(tc.tile_pool(name="const", bufs=1))
    win_pool = ctx.enter_context(tc.tile_pool(name="win", bufs=1))
    u_pool = ctx.enter_context(tc.tile_pool(name="u", bufs=6))
    psum_pool = ctx.enter_context(tc.tile_pool(name="psum", bufs=8, space="PSUM"))

    stat = []
    for d in range(3):
        st_t = const_pool.tile([KP, MP], f16, name=f"stat{d}")
        nc.sync.dma_start(st_t[:], s_ap[d])
        stat.append(st_t)

    WIN = [
        win_pool.tile([KP, NB * TF], f16, tag=f"W{img}", name=f"W{img}")
        for img in range(n_img)
    ]

    def wslice(img, b, p0, p1, t0, t1):
        return WIN[img][p0:p1, b * TF + t0 : b * TF + t1]

    qrr = [nc.sync, nc.scalar, nc.gpsimd]

    # ── initial load: contiguous chunks of LG blocks ──
    lg = min(LG, NB)
    for img in range(n_img):
        for i, b0 in enumerate(range(0, NB, lg)):
            eng = qrr[i % 3]
            eng.dma_start(
                WIN[img][:, b0 * TF : (b0 + lg) * TF],
                xw_ap[img, :, b0 : b0 + lg, :],
            )

    # ── steps ──
    for s in range(steps):
        last = s == steps - 1
        for img in range(n_img):
            for b in range(NB):
                psum = psum_pool.tile([MP, T], f32, tag="ps")
                for d in range(3):
                    nc.tensor.matmul(
                        psum[:],
                        stat[d][:],
                        wslice(img, b, 0, KP, d, d + T),
                        start=(d == 0),
                        stop=(d == 2),
                    )
                u = u_pool.tile([MP, T], f16, tag="u")
                nc.scalar.activation(u[:], psum[:], Relu)
                nc.vector.tensor_add(
                    wslice(img, b, 0, MP, T_OF, T_OF + T),
                    wslice(img, b, 0, MP, T_OF, T_OF + T),
                    u[:],
                )
                nc.gpsimd.tensor_scalar_min(
                    wslice(img, b, 0, MP, T_OF, T_OF + T),
                    wslice(img, b, 0, MP, T_OF, T_OF + T),
                    1.0,
                )
                if not last:
                    nc.gpsimd.tensor_copy(
                        wslice(img, b, 0, MP, 0, 1), wslice(img, b, 0, MP, T, T + 1)
                    )
                    nc.gpsimd.tensor_copy(
                        wslice(img, b, 0, MP, T + 1, T + 2),
                        wslice(img, b, 0, MP, T_OF, T_OF + 1),
                    )
                    if b % HG == HG - 1:
                        # batched halo exchange for blocks [lo..b]; b=0's
                        # wrap partners are handled at end of step
                        lo = max(b - HG + 1, 1)
                        w = WIN[img]
                        # UP: rows 0,1 of blocks lo..b -> bottom halos of lo-1..b-1
                        nc.sync.dma_start(
                            w[108:120, (lo - 1) * TF : b * TF],
                            w[0:12, lo * TF : (b + 1) * TF],
                        )
                        # DOWN: rows 14,15 of lo-1..b-1 -> top halos of lo..b
                        nc.sync.dma_start(
                            w[96:108, lo * TF : (b + 1) * TF],
                            w[84:96, (lo - 1) * TF : b * TF],
                        )
                else:
                    if b % SG == SG - 1:
                        lo = b - SG + 1
                        eng = qrr[(b // SG) % 3]
                        eng.dma_start(
                            yw_ap[img, :, lo : b + 1, :],
                            WIN[img]
                            .rearrange("p (b t) -> p b t", b=NB)[
                                0:MP, lo : b + 1, T_OF : T_OF + T
                            ],
                        )
            if not last:
                # circular wrap halos, after the whole image's step
                w = WIN[img]
                e = NB - 1
                nc.scalar.dma_start(w[96:108, 0:TF], w[84:96, e * TF : (e + 1) * TF])
                nc.scalar.dma_start(w[108:120, e * TF : (e + 1) * TF], w[0:12, 0:TF])

    ctx.close()


_PROGRAM_CACHE = {}


def _build_program(n_img, H, Wc, steps, n_cores=N_CORES):
    key = (n_img, H, Wc, steps, n_cores)
    if key in _PROGRAM_CACHE:
        return _PROGRAM_CACHE[key]
    import concourse.tile as tile
    from concourse import bacc, mybir

    nc = bacc.Bacc(
        "TRN2",
        target_bir_lowering=False,
        debug=False,
        enable_asserts=False,
        num_devices=n_cores,
    )
    f16 = mybir.dt.float16
    NB, T = H // R, Wc // 2
    xw_ap = nc.dram_tensor("xw", (n_img, KP, NB, T + 2), f16, kind="ExternalInput").ap()
    s_ap = nc.dram_tensor("S", (3, KP, MP), f16, kind="ExternalInput").ap()
    yw_ap = nc.dram_tensor("yw", (n_img, MP, NB, T), f16, kind="ExternalOutput").ap()
    with tile.TileContext(nc) as tc:
        build_body(tc, xw_ap, s_ap, yw_ap, n_img, H, Wc, steps)
    nc.compile()
    _PROGRAM_CACHE[key] = nc
    return nc


def kernel(x: np.ndarray, W: np.ndarray, steps) -> np.ndarray:
    from concourse.bass_utils import run_bass_kernel_spmd

    x = np.ascontiguousarray(np.asarray(x), dtype=np.float32)
    W = np.asarray(W, dtype=np.float32)
    steps = int(steps)
    n, c, H, Wc = x.shape
    assert c == 3 and n % N_CORES == 0
    per = n // N_CORES

    nc = _build_program(per, H, Wc, steps)
    S = make_stationaries(W)
    xw = pack_x(x)
    in_maps = [
        {"xw": xw[i * per : (i + 1) * per], "S": S} for i in range(N_CORES)
    ]
    res = run_bass_kernel_spmd(nc, in_maps, core_ids=list(range(N_CORES)))
    yw = np.concatenate([res.results[i]["yw"] for i in range(N_CORES)], axis=0)
    return unpack_y(yw, H, Wc)


# revision 19
# speedup vs baseline: 10.7133x; 10.7133x over previous
"""Trainium2 Bass kernel for nn_CAKernel_47459388621075.

10 steps of x = clip(x + 0.1*relu(conv5x5_circular(x, W)), 0, 1) on
x:(16,3,1024,1024) f32, W:(3,3,5,5) f32.

Sharding: batch-parallel over 8 NeuronCores (2 images/core); the circular
conv is per-image, so no cross-core communication.

Per-core: the whole state lives in SBUF as fp16 for all 10 steps — zero HBM
traffic between steps. Each image is one SBUF tile WIN = [120, 64*514]: 64
row-blocks of R=16 rows, block b occupying free columns [b*514, (b+1)*514)
= column-pair index t in [-1, 513) (1-pair circular halo each side).
Partitions pack (row, channel, column-parity):
  rows v in [0,16):  p = 6v + 2ci + s            (main, [0:96))
  rows -2,-1:        p = 96 + 6(v+2) + 2ci + s   (top halo, [96:108))
  rows 16,17:        p = 108 + 6(v-16) + 2ci + s (bottom halo, [108:120))
Main-first ordering keeps every compute op based at partition 0 (engines
require quadrant-aligned partition bases); halo partitions are written by
DMA only (partition moves need the DMA crossbar; compute engines are
lane-aligned; strided-element DMAs blow the descriptor budget, which is
why the hosts packs/unpacks the pair layout and device I/O is contiguous).

Packing column pairs into partitions doubles the useful work per streamed
PE column: the 5x5x3x3 conv becomes 3 PSUM-accumulated fp16 matmuls per
block (pair-shift delta in {-1,0,+1}) with stationaries S_d[120,96] that
encode (ci,dy,dx,parity) -> (co,r,parity').

Post-conv per block: ACT computes u = relu(psum) (0.1 folded into the
stationaries), DVE adds it into the window in place, Pool clips with
min(.,1) and refreshes the two circular column-pair halo columns. The
2-row halos ride to the neighbour blocks in batched 12-partition DMAs (one
UP + one DOWN DMA per HG consecutive blocks — all windows of an image are
one tile, so the per-block copies merge into a single affine transfer).
Halo DMAs into a window are emitted after that window's matmuls of the
current step, so the step still reads the old halo (Tile's WAR tracking
orders the write after those reads).
"""
import sys

sys.path.insert(0, "/opt/trn_rl_repo")

import numpy as np

N_CORES = 8
R = 16            # output rows per block
KP = 120          # window partitions
MP = 96           # psum partitions
T_OF = 1          # free offset of pair 0 inside a block's window
HG = 4            # halo-DMA batch (blocks per UP/DOWN transfer)
SG = 4            # store batch
LG = 8            # load batch


def _p_of(ci, v, s):
    if 0 <= v < R:
        return 6 * v + 2 * ci + s
    if v < 0:
        return 96 + 6 * (v + 2) + 2 * ci + s
    return 108 + 6 * (v - R) + 2 * ci + s


def make_stationaries(W: np.ndarray) -> np.ndarray:
    """S[d, p, m] for pair-shift d-1 in {-1,0,+1}; 0.1 scale folded in."""
    assert W.shape == (3, 3, 5, 5)
    S = np.zeros((3, KP, MP), dtype=np.float32)
    for co in range(3):
        for ci in range(3):
            for r in range(R):
                for dy in range(5):
                    v = r + dy - 2
                    for dx in range(5):
                        for sp in range(2):
                            tot = sp + dx - 2
                            d = tot >> 1
                            s = tot & 1
                            p = _p_of(ci, v, s)
                            m = 6 * r + 2 * co + sp
                            S[d + 1, p, m] += 0.1 * W[co, ci, dy, dx]
    return S.astype(np.float16)


def pack_x(x: np.ndarray) -> np.ndarray:
    """(n,3,H,W) f32 -> (n, KP, NB, TF) f16 in window layout (p-major)."""
    n, c, H, Wc = x.shape
    NB, T = H // R, Wc // 2
    TF = T + 2
    x16 = x.astype(np.float16)
    xw = np.empty((n, KP, NB, TF), np.float16)
    boff = R * np.arange(NB)
    for w in range(20):
        v = w if w < 16 else (w - 18 if w < 18 else w - 2)  # w16,17 -> -2,-1; w18,19 -> 16,17
        rows = (boff + v) % H
        for ci in range(3):
            for s in range(2):
                p = 6 * w + 2 * ci + s if w < 16 else (
                    96 + 6 * (w - 16) + 2 * ci + s if w < 18 else 108 + 6 * (w - 18) + 2 * ci + s
                )
                xw[:, p, :, T_OF : T_OF + T] = x16[:, ci, rows, s::2]
    xw[:, :, :, 0] = xw[:, :, :, T]
    xw[:, :, :, T + 1] = xw[:, :, :, T_OF]
    return xw


def unpack_y(yw: np.ndarray, H: int, Wc: int) -> np.ndarray:
    """(n, MP, NB, T) f16 window layout -> (n,3,H,W) f32."""
    n = yw.shape[0]
    NB, T = H // R, Wc // 2
    y = np.empty((n, 3, H, Wc), np.float32)
    rows = (R * np.arange(NB)[:, None] + np.arange(R)[None, :])  # (NB, R)
    for v in range(R):
        for ci in range(3):
            for s in range(2):
                p = 6 * v + 2 * ci + s
                y[:, ci, rows[:, v], s::2] = yw[:, p].astype(np.float32)
    return y


def build_body(tc, xw_ap, s_ap, yw_ap, n_img, H, Wc, steps):
    from contextlib import ExitStack

    from concourse import mybir

    nc = tc.nc
    f32 = mybir.dt.float32
    f16 = mybir.dt.float16
    Relu = mybir.ActivationFunctionType.Relu

    NB = H // R
    T = Wc // 2
    TF = T + 2
    assert H % R == 0 and Wc % 2 == 0 and NB >= 3 and NB % HG == 0

    ctx = ExitStack()
    const_pool = ctx.enter_context(tc.tile_pool(name="const", bufs=1))
    win_pool = ctx.enter_context(tc.tile_pool(name="win", bufs=1))
    u_pool = ctx.enter_context(tc.tile_pool(name="u", bufs=6))
    psum_pool = ctx.enter_context(tc.tile_pool(name="psum", bufs=8, space="PSUM"))

    stat = []
    for d in range(3):
        st_t = const_pool.tile([KP, MP], f16, name=f"stat{d}")
        nc.sync.dma_start(st_t[:], s_ap[d])
        stat.append(st_t)

    WIN = [
        win_pool.tile([KP, NB * TF], f16, tag=f"W{img}", name=f"W{img}")
        for img in range(n_img)
    ]

    def wslice(img, b, p0, p1, t0, t1):
        return WIN[img][p0:p1, b * TF + t0 : b * TF + t1]

    qrr = [nc.sync, nc.scalar, nc.gpsimd]

    # ── initial load: contiguous chunks of LG blocks ──
    lg = min(LG, NB)
    for img in range(n_img):
        for i, b0 in enumerate(range(0, NB, lg)):
            eng = qrr[i % 3]
            eng.dma_start(
                WIN[img][:, b0 * TF : (b0 + lg) * TF],
                xw_ap[img, :, b0 : b0 + lg, :],
            )

    # ── steps ──
    # Per group of HG=4 blocks: 12 matmuls (PE), 4 relu (ACT), ONE grouped
    # add + ONE grouped min (DVE, multi-block free AP amortizes the per-op
    # overhead), 8 tiny wrap-column copies (GpSimd), 2 batched halo DMAs (SP).
    assert NB % HG == 0
    for s in range(steps):
        last = s == steps - 1
        for img in range(n_img):
            wview = WIN[img].rearrange("p (b t) -> p b t", b=NB)
            for g in range(NB // HG):
                b0 = g * HG
                ug = u_pool.tile([MP, HG * T], f16, tag="u")
                for i in range(HG):
                    b = b0 + i
                    psum = psum_pool.tile([MP, T], f32, tag="ps")
                    for d in range(3):
                        nc.tensor.matmul(
                            psum[:],
                            stat[d][:],
                            wslice(img, b, 0, KP, d, d + T),
                            start=(d == 0),
                            stop=(d == 2),
                        )
                    nc.scalar.activation(ug[:, i * T : (i + 1) * T], psum[:], Relu)
                wv = wview[0:MP, b0 : b0 + HG, T_OF : T_OF + T]
                uv = ug.rearrange("p (b t) -> p b t", b=HG)
                nc.vector.tensor_add(wv, wv, uv)
                nc.vector.tensor_scalar_min(wv, wv, 1.0)
                if not last:
                    for i in range(HG):
                        b = b0 + i
                        nc.gpsimd.tensor_copy(
                            wslice(img, b, 0, MP, 0, 1),
                            wslice(img, b, 0, MP, T, T + 1),
                        )
                        nc.gpsimd.tensor_copy(
                            wslice(img, b, 0, MP, T + 1, T + 2),
                            wslice(img, b, 0, MP, T_OF, T_OF + 1),
                        )
                    # batched halo exchange for blocks [lo..b0+HG-1]; block
                    # 0's wrap partners are handled at end of step
                    lo = max(b0, 1)
                    hi = b0 + HG - 1
                    w = WIN[img]
                    # UP: rows 0,1 of blocks lo..hi -> bottom halos of lo-1..hi-1
                    nc.sync.dma_start(
                        w[108:120, (lo - 1) * TF : hi * TF],
                        w[0:12, lo * TF : (hi + 1) * TF],
                    )
                    # DOWN: rows 14,15 of lo-1..hi-1 -> top halos of lo..hi
                    nc.sync.dma_start(
                        w[96:108, lo * TF : (hi + 1) * TF],
                        w[84:96, (lo - 1) * TF : hi * TF],
                    )
                else:
                    eng = qrr[g % 3]
                    eng.dma_start(
                        yw_ap[img, :, b0 : b0 + HG, :],
                        wview[0:MP, b0 : b0 + HG, T_OF : T_OF + T],
                    )
            if not last:
                # circular wrap halos, after the whole image's step
                w = WIN[img]
                e = NB - 1
                nc.scalar.dma_start(w[96:108, 0:TF], w[84:96, e * TF : (e + 1) * TF])
                nc.scalar.dma_start(w[108:120, e * TF : (e + 1) * TF], w[0:12, 0:TF])

    ctx.close()


_PROGRAM_CACHE = {}


def _build_program(n_img, H, Wc, steps, n_cores=N_CORES):
    key = (n_img, H, Wc, steps, n_cores)
    if key in _PROGRAM_CACHE:
        return _PROGRAM_CACHE[key]
    import concourse.tile as tile
    from concourse import bacc, mybir

    nc = bacc.Bacc(
        "TRN2",
        target_bir_lowering=False,
        debug=False,
        enable_asserts=False,
        num_devices=n_cores,
    )
    f16 = mybir.dt.float16
    NB, T = H // R, Wc // 2
    xw_ap = nc.dram_tensor("xw", (n_img, KP, NB, T + 2), f16, kind="ExternalInput").ap()
    s_ap = nc.dram_tensor("S", (3, KP, MP), f16, kind="ExternalInput").ap()
    yw_ap = nc.dram_tensor("yw", (n_img, MP, NB, T), f16, kind="ExternalOutput").ap()
    with tile.TileContext(nc) as tc:
        build_body(tc, xw_ap, s_ap, yw_ap, n_img, H, Wc, steps)
    nc.compile()
    _PROGRAM_CACHE[key] = nc
    return nc


def kernel(x: np.ndarray, W: np.ndarray, steps) -> np.ndarray:
    from concourse.bass_utils import run_bass_kernel_spmd

    x = np.ascontiguousarray(np.asarray(x), dtype=np.float32)
    W = np.asarray(W, dtype=np.float32)
    steps = int(steps)
    n, c, H, Wc = x.shape
    assert c == 3 and n % N_CORES == 0
    per = n // N_CORES

    nc = _build_program(per, H, Wc, steps)
    S = make_stationaries(W)
    xw = pack_x(x)
    in_maps = [
        {"xw": xw[i * per : (i + 1) * per], "S": S} for i in range(N_CORES)
    ]
    res = run_bass_kernel_spmd(nc, in_maps, core_ids=list(range(N_CORES)))
    yw = np.concatenate([res.results[i]["yw"] for i in range(N_CORES)], axis=0)
    return unpack_y(yw, H, Wc)
